# revision 1
# baseline (speedup 1.0000x reference)
"""Bass/Tile Trainium2 kernel for the 2-layer FC-LSTM + Dense model.

Strategy (data-parallel over batch, 8 cores x 32 samples):
  - Transposed on-chip layout: feature dim on the 128 SBUF partitions,
    (time x batch) on the free dim.
  - No separate input-GEMM phase: the x*W0 contribution for step t+1 is
    matmul-accumulated directly into that step's PSUM bank during round t
    (PE cost is identical to a chunked GEMM -- matmul time is purely
    output-columns -- but this removes the identity-prefill matmuls, all
    PSUM->SBUF zx evacuation copies, and the GEMM warm-up head).
  - Per-step PSUM bank: [x*W0 prefill MMs (emitted a round early, off the
    serial chain)] + [U*h MMs (chain-gated)].  L1 likewise prefills
    W1*h0(t)+b1 a round early; only its 16 U1 matmuls are chain-gated.
  - Gate math in bf16 (DVE 2x mode), cell state bf16 (verified 7.2e-3 rel
    err vs 2e-2 budget).  Gate columns host-permuted [i f o g] so one
    sigmoid covers a contiguous 6-chunk block.
  - z PSUM tiles padded to a full 2KB bank so next step's prefill (PE
    write) never shares a bank with this step's sigmoid/DVE reads.
  - Dense output (Wd) sliced into (half-chunk, m) units of 2 matmuls +
    1 biased evacuation, drained one unit per round to keep PE/ACT load
    flat.
"""

import numpy as np
import ml_dtypes
from contextlib import ExitStack

import concourse.bass as bass
import concourse.mybir as mybir
import concourse.tile as tile
from concourse.tile_rust import add_dep_helper
from concourse import bacc, bass_utils

# problem constants (hardcoded per contract)
B, N, T, F_IN = 256, 300, 64, 3
U_DIM = 256
G = 4 * U_DIM              # 1024 gates per layer
F_OUT = 2
D_IN = N * F_IN            # 900
D_OUT = N * F_OUT          # 600
NCORES = 8
BL = B // NCORES           # 32 batch rows per core
NTOK = T * BL              # 2048 tokens per core (token id = t*BL + b)
KP = 1024                  # padded input-feature dim; row 900 is the bias row
KT_IN = KP // 128          # 8 k-tiles for the input contribution
GC = G // 128              # 8 gate chunks
HC = U_DIM // 128          # 2 hidden chunks
MT = 5                     # output m-tiles (600 -> 640)
D_OUT_PAD = MT * 128
QS = 8                     # steps per dense-output half-chunk
QN = QS * BL               # 256 tokens
NQ = T // QS               # 8 half-chunks
LAG = 2

BF16 = mybir.dt.bfloat16
F32 = mybir.dt.float32
NP_BF16 = ml_dtypes.bfloat16
AF = mybir.ActivationFunctionType
ALU = mybir.AluOpType

# keras gate order i,f,g,o -> our chunk order i,f,o,g
_PERM = np.concatenate([
    np.arange(0, U_DIM),                  # i
    np.arange(U_DIM, 2 * U_DIM),          # f
    np.arange(3 * U_DIM, 4 * U_DIM),      # o
    np.arange(2 * U_DIM, 3 * U_DIM),      # g
])


def _w_tiles(Wp, kt):
    """(kt*128, GC*128) f32 -> (128, kt, GC, 128) bf16 lhsT tile array."""
    return np.ascontiguousarray(
        Wp.astype(NP_BF16).reshape(kt, 128, GC, 128).transpose(1, 0, 2, 3))


def _prep_shared(W0, U0, b0, W1, U1, b1, Wd, bd):
    W0p = np.zeros((KP, G), np.float32)
    W0p[:D_IN] = W0[:, _PERM]
    W0p[D_IN] = b0[_PERM]
    w0t = _w_tiles(W0p, KT_IN)
    u0t = _w_tiles(U0[:, _PERM], HC)
    w1t = _w_tiles(W1[:, _PERM], HC)
    u1t = _w_tiles(U1[:, _PERM], HC)
    Wdp = np.zeros((U_DIM, D_OUT_PAD), np.float32)
    Wdp[:, :D_OUT] = Wd
    wdt = np.ascontiguousarray(
        Wdp.astype(NP_BF16).reshape(HC, 128, MT, 128).transpose(1, 0, 2, 3))
    b1mm = np.zeros((128, GC, 128), np.float32)
    b1mm[D_IN % 128] = b1[_PERM].reshape(GC, 128)   # rank-1 bias vs xT's ones row
    b1mm = b1mm.astype(NP_BF16)
    bdp = np.zeros(D_OUT_PAD, np.float32)
    bdp[:D_OUT] = bd
    bdt = np.ascontiguousarray(bdp.reshape(MT, 128).T)
    return dict(w0t=w0t, u0t=u0t, w1t=w1t, u1t=u1t, wdt=wdt, b1mm=b1mm, bdt=bdt)


def _prep_x(x_core):
    """(BL, N, T, F_IN) f32 -> (128, KT_IN, NTOK) bf16 with bias ones-row."""
    seq = x_core.transpose(0, 2, 1, 3).reshape(BL, T, D_IN)   # (b, t, feat)
    xT = np.zeros((KP, T, BL), np.float32)
    xT[:D_IN] = seq.transpose(2, 1, 0)                        # (feat, t, b)
    xT[D_IN] = 1.0
    return np.ascontiguousarray(
        xT.astype(NP_BF16).reshape(KT_IN, 128, NTOK).transpose(1, 0, 2))


class _LstmStepper:
    """Per-step emitters for one LSTM layer.

    The z(t) PSUM bank is built in two pieces: the input contribution
    (x*W0 resp. W1*h0+b1) is matmul-prefilled a round EARLY (its operands
    are ready long in advance, so these MMs fill the PE while the previous
    step's gate math runs), then the U matmuls -- the only chain-gated PE
    work -- accumulate on top."""

    def __init__(self, nc, work, ps_r, u, hseq, cst, lname,
                 w_in=None, xT=None, hprev=None, bias_mm=None, ones_row=None,
                 h_engine=None, h_ksplit=False):
        self.nc, self.work, self.ps_r = nc, work, ps_r
        self.u, self.hseq, self.cst = u, hseq, cst
        self.ln = lname
        self.w_in, self.xT = w_in, xT
        self.hprev, self.bias_mm, self.ones_row = hprev, bias_mm, ones_row
        self.h_engine = h_engine or nc.vector
        self.h_ksplit = h_ksplit
        self.ps_by_t = {}
        self.sfo = None
        self.sfoo = None
        self.sig_o_inst = None

    def emit_prefill(self, t, after=None):
        nc = self.nc

        def dep(inst):
            if after is not None:
                add_dep_helper(inst.ins, after.ins, sync=False,
                               reason="prefill fills the sig/DVE window, "
                                      "never ahead of chain-critical U MMs")
            return inst

        # full 2KB bank per buffer: PE prefill of step t+1 must never share
        # a bank with step t's ACT/DVE reads (PSUM bank collisions serialize)
        pst = self.ps_r.tile([128, 2, GC, BL], F32, tag=f"{self.ln}ps",
                             name=f"{self.ln}_ps_{t}", bufs=2)
        ps = pst[:, 0]
        self.ps_by_t[t] = ps
        last = t == 0   # t=0 has no U matmuls: close the group here
        if self.xT is not None:
            # k-major so the first-arriving w0 k-tile DMA unblocks 8 MMs;
            # per-element has_written makes any accumulation order valid with
            # a single start=True on the very first MM of the bank
            for k in range(KT_IN):
                for g in range(GC):
                    dep(nc.tensor.matmul(
                        ps[:, g, :], self.w_in[:, k, g, :],
                        self.xT[:, k, t * BL:(t + 1) * BL],
                        start=(g == 0 and k == 0),
                        stop=(last and g == GC - 1 and k == KT_IN - 1)))
        else:
            for k in range(HC):
                for g in range(GC):
                    dep(nc.tensor.matmul(
                        ps[:, g, :], self.w_in[:, k, g, :], self.hprev[:, t, k, :],
                        start=(g == 0 and k == 0), stop=False))
            for g in range(GC):
                dep(nc.tensor.matmul(
                    ps[:, g, :], self.bias_mm[:, g, :], self.ones_row(t),
                    start=False, stop=(last and g == GC - 1)))

    def emit_u(self, t):
        nc = self.nc
        ps = self.ps_by_t[t]
        inst = None
        for g in range(GC):
            for k in range(HC):
                inst = nc.tensor.matmul(
                    ps[:, g, :], self.u[:, k, g, :], self.hseq[:, t - 1, k, :],
                    start=False, stop=(g == GC - 1 and k == HC - 1))
        return inst

    def emit_sig(self, t, after=None, split=False):
        ps = self.ps_by_t[t]
        self.split = split
        if split:
            # i,f only; o is emitted AFTER the t1/mul/add cell ops so Tile's
            # transitive wait-coalescing gives t1 a tight ACT gate (sig_if)
            # and a tight PE gate (the U matmuls), not sig_o / prefills
            sfo = self.work.tile([128, 4, BL], BF16, tag=f"{self.ln}sfo",
                                 name=f"{self.ln}_sfo_{t}")
            inst = self.nc.scalar.activation(sfo[:], ps[:, 0:4, :], AF.Sigmoid)
        else:
            sfo = self.work.tile([128, 6, BL], BF16, tag=f"{self.ln}sfo",
                                 name=f"{self.ln}_sfo_{t}")
            inst = self.nc.scalar.activation(sfo[:], ps[:, 0:6, :], AF.Sigmoid)
        if after is not None:
            add_dep_helper(inst.ins, after.ins, sync=False,
                           reason="keep chain-critical sigmoid first on ACT")
        self.sfo = sfo
        self.sfoo = None
        return inst

    def emit_cell_pre(self, t, after=None):
        nc, sfo = self.nc, self.sfo
        ps = self.ps_by_t[t]

        def dep(inst):
            if after is not None:
                add_dep_helper(inst.ins, after.ins, sync=False,
                               reason="keep L0 cell block contiguous on DVE")
            return inst

        cnew = self.work.tile([128, 2, BL], BF16, tag=f"{self.ln}cst",
                              name=f"{self.ln}_cst_{t}", bufs=2)
        if t == 0:
            dep(nc.vector.scalar_tensor_tensor(
                cnew[:], ps[:, 6:8, :], 0.0, sfo[:, 0:2, :],
                op0=ALU.max, op1=ALU.mult))
        else:
            t1 = self.work.tile([128, 2, BL], BF16, tag=f"{self.ln}t1",
                                name=f"{self.ln}_t1_{t}")
            c2 = self.work.tile([128, 2, BL], BF16, tag=f"{self.ln}c2",
                                name=f"{self.ln}_c2_{t}")
            dep(nc.vector.scalar_tensor_tensor(
                t1[:], ps[:, 6:8, :], 0.0, sfo[:, 0:2, :],
                op0=ALU.max, op1=ALU.mult))
            if self.split:
                # sig_o emitted here (after t1, before mul): its coalesced
                # DVE wait then points at t1, so it fires ~2 ops earlier and
                # the h-mul is no longer sig_o-gated
                sfoo = self.work.tile([128, 2, BL], BF16,
                                      tag=f"{self.ln}sfoo",
                                      name=f"{self.ln}_sfoo_{t}")
                self.sig_o_inst = self.nc.scalar.activation(
                    sfoo[:], ps[:, 4:6, :], AF.Sigmoid)
                self.sfoo = sfoo
            dep(nc.vector.tensor_mul(c2[:], self.cst[:], sfo[:, 2:4, :]))
            dep(nc.vector.tensor_add(cnew[:], c2[:], t1[:]))
        self.cst = cnew

    def emit_cell_h(self, t, h_engine=None):
        h_engine = h_engine or self.h_engine
        cnew, ps = self.cst, self.ps_by_t.pop(t)
        if self.split and self.sfoo is not None:
            o_ap = self.sfoo[:]
        elif self.split:
            sfoo = self.work.tile([128, 2, BL], BF16, tag=f"{self.ln}sfoo",
                                  name=f"{self.ln}_sfoo_{t}")
            self.sig_o_inst = self.nc.scalar.activation(
                sfoo[:], ps[:, 4:6, :], AF.Sigmoid)
            o_ap = sfoo[:]
        else:
            self.sig_o_inst = None
            o_ap = self.sfo[:, 4:6, :]
        # c >= 0 by construction, so relu(c) is a no-op: h = o * c
        return h_engine.tensor_mul(self.hseq[:, t, :, :], cnew[:], o_ap)


def build_nc(edge_start=None, sig_split=True, lag=LAG, h_ksplit=False):
    nc = bacc.Bacc("TRN2", target_bir_lowering=False, debug=False)
    xT_d = nc.dram_tensor("xT", (128, KT_IN, NTOK), BF16, kind="ExternalInput").ap()
    w0_d = nc.dram_tensor("w0t", (128, KT_IN, GC, 128), BF16, kind="ExternalInput").ap()
    u0_d = nc.dram_tensor("u0t", (128, HC, GC, 128), BF16, kind="ExternalInput").ap()
    w1_d = nc.dram_tensor("w1t", (128, HC, GC, 128), BF16, kind="ExternalInput").ap()
    u1_d = nc.dram_tensor("u1t", (128, HC, GC, 128), BF16, kind="ExternalInput").ap()
    wd_d = nc.dram_tensor("wdt", (128, HC, MT, 128), BF16, kind="ExternalInput").ap()
    b1_d = nc.dram_tensor("b1mm", (128, GC, 128), BF16, kind="ExternalInput").ap()
    bd_d = nc.dram_tensor("bdt", (128, MT), F32, kind="ExternalInput").ap()
    out_d = nc.dram_tensor("out", (128, MT, NTOK), F32, kind="ExternalOutput").ap()

    with tile.TileContext(nc) as tc, ExitStack() as ctx:
        const = ctx.enter_context(tc.tile_pool(name="const", bufs=1))
        xT = const.tile([128, KT_IN, NTOK], BF16)
        w0 = const.tile([128, KT_IN, GC, 128], BF16)
        u0 = const.tile([128, HC, GC, 128], BF16)
        w1 = const.tile([128, HC, GC, 128], BF16)
        u1 = const.tile([128, HC, GC, 128], BF16)
        wd = const.tile([128, HC, MT, 128], BF16)
        b1mm = const.tile([128, GC, 128], BF16)
        bds = const.tile([128, MT], F32)
        h0 = const.tile([128, T, HC, BL], BF16)
        h1 = const.tile([128, T, HC, BL], BF16)
        c0 = const.tile([128, HC, BL], BF16)
        c1 = const.tile([128, HC, BL], BF16)
        warm = const.tile([128, 2], F32)

        # pull the two ACT table loads (sigmoid / identity sets) to t=0 so
        # they run under the input DMA instead of delaying sig(0)
        nc.vector.memset(warm[:], 0.0)
        nc.scalar.activation(warm[:, 0:1], warm[:, 1:2], AF.Sigmoid)
        nc.scalar.activation(warm[:, 0:1], warm[:, 1:2], AF.Identity)

        # need-ordered input DMA across the issue queues: the first steps'
        # tokens and w0 gate-tiles first (prefill(0) starts a ~2us in), then
        # recurrence/L1 weights, then the bulk of the sequence + dense tail.
        # All transfers are contiguous intervals of their SBUF tiles (per-k
        # planes for xT, k-pairs for w0) so the subtile tracker sees exact
        # ranges.  Everything goes through the two HW queues in need-order:
        # the modeled DMA pipe is a single serial device, so queue-jumping
        # SWDGE bulk would delay the critical w0/xT-head transfers.
        # Everything rides the SP queue in strict need-order (the modeled DMA
        # pipe is one serial FIFO device, and ACT must stay free for the
        # chain-critical sigmoids + its table loads).  First: just step 0's
        # tokens + w0 so the pipeline starts ~7us in, then data in
        # consumption order.
        # few, large transfers: each dma_start costs ~650ns of SP issue time,
        # so the first handful of issues set the head timeline.  The 3D
        # token-range transfers produce interval-merged (conservative) deps
        # that happen to match consumption order exactly.
        nc.sync.dma_start(out=xT[:, :, 0:256], in_=xT_d[:, :, 0:256])
        for k in range(0, KT_IN, 2):
            nc.sync.dma_start(out=w0[:, k:k + 2], in_=w0_d[:, k:k + 2])
        nc.sync.dma_start(out=u0[:], in_=u0_d[:])
        nc.sync.dma_start(out=w1[:], in_=w1_d[:])
        nc.sync.dma_start(out=u1[:], in_=u1_d[:])
        nc.sync.dma_start(out=b1mm[:], in_=b1_d[:])
        nc.sync.dma_start(out=xT[:, :, 256:512], in_=xT_d[:, :, 256:512])
        nc.sync.dma_start(out=wd[:], in_=wd_d[:])
        nc.sync.dma_start(out=bds[:], in_=bd_d[:])
        nc.sync.dma_start(out=xT[:, :, 512:1024], in_=xT_d[:, :, 512:1024])
        nc.sync.dma_start(out=xT[:, :, 1024:NTOK], in_=xT_d[:, :, 1024:NTOK])

        ps_r = ctx.enter_context(tc.tile_pool(name="ps_r", bufs=2, space="PSUM"))
        ps_g = ctx.enter_context(tc.tile_pool(name="ps_g", bufs=3, space="PSUM"))
        work = ctx.enter_context(tc.tile_pool(name="work", bufs=3))
        # one buffer per dense-output unit: an evac must never wait on an
        # out-DMA completion to recycle a buffer (it would drag the DMA
        # pipeline into ACT's in-order stream, ahead of the sigmoids)
        outp = ctx.enter_context(tc.tile_pool(name="outp", bufs=NQ * MT))

        st0 = _LstmStepper(nc, work, ps_r, u0, h0, c0, "l0", w_in=w0, xT=xT,
                           h_ksplit=h_ksplit)
        st1 = _LstmStepper(nc, work, ps_r, u1, h1, c1, "l1",
                           w_in=w1, hprev=h0, bias_mm=b1mm,
                           ones_row=lambda t: xT[:, KT_IN - 1, t * BL:(t + 1) * BL],
                           h_engine=nc.vector)

        # dense-output work units: (half-chunk, m) -> 2 MMs in round r, then
        # the biased evacuation + DMA one round LATER.  Evacuating in the
        # same round puts an ACT instruction with a deep PE-counter wait in
        # front of the next sigmoids in ACT's in-order stream, gating the
        # whole recurrence cadence on the filler matmuls; a round later the
        # wait is long resolved when the evac reaches the queue head.
        p5q = []
        p5ev = []

        def emit_p5_mm(lo, ntok, m, after_mm=None):
            ps5 = ps_g.tile([128, 512], F32, tag="p5", name=f"p5_{lo}_{m}",
                            bufs=4)
            t0, ns = lo // BL, ntok // BL
            for k in range(HC):
                mm = nc.tensor.matmul(
                    ps5[:, 0:ntok], wd[:, k, m, :], h1[:, t0:t0 + ns, k, :],
                    start=(k == 0), stop=(k == HC - 1))
                if after_mm is not None:
                    add_dep_helper(mm.ins, after_mm.ins, sync=False,
                                   reason="dense MMs fill the sig/DVE window")
            p5ev.append((lo, ntok, m, ps5))

        def emit_p5_ev(lo, ntok, m, ps5, after=None):
            ev = nc.scalar.activation(ot := outp.tile(
                [128, ntok], F32, tag="ot", name=f"ot{lo}_{m}"),
                ps5[:, 0:ntok], AF.Identity, bias=bds[:, m:m + 1])
            if after is not None:
                add_dep_helper(ev.ins, after.ins, sync=False,
                               reason="dense evac after chain sigmoids")
            nc.sync.dma_start(out=out_d[:, m, lo:lo + ntok], in_=ot[:])

        st0.emit_prefill(0)
        for r in range(T + lag):
            a = r if r < T else None
            b = r - lag if 0 <= r - lag < T else None
            # chain-gated recurrence matmuls first in the PE queue; all other
            # PE work this round is edge-ordered after them so the in-order
            # PE stream reaches U0 the moment h0 lands.  During the DMA-gated
            # warm-up rounds the scheduler floats prefills freely.
            u0i = st0.emit_u(a) if a is not None and a > 0 else None
            gate = u0i if edge_start is not None and r >= edge_start else None
            sig0 = None
            h0i = None
            if a is not None:
                sig0 = st0.emit_sig(a, split=sig_split)
                st0.emit_cell_pre(a)
                h0i = st0.emit_cell_h(a)
            sig1 = None
            if b is not None and b > 0:
                st1.emit_u(b)
            if b is not None:
                # sig1 must not slot between sig_if and sig_o on ACT: h0
                # waits on sig_o, so sig_o goes right after sig_if
                sig1 = st1.emit_sig(b, after=(st0.sig_o_inst or sig0))
                st1.emit_cell_pre(b, after=h0i)
                st1.emit_cell_h(
                    b, h_engine=nc.vector if a is None else None)
            # next-step input prefills fill the PE during this round's tail
            if a is not None and a + 1 < T:
                st0.emit_prefill(a + 1, after=gate)
            nb = r - lag + 1
            if 0 <= nb < T:
                st1.emit_prefill(nb, after=gate)
            # deferred dense evacs: their PE waits resolved last round
            ev_gate = sig1 or sig0
            for unit in p5ev[:]:
                p5ev.remove(unit)
                emit_p5_ev(*unit, after=ev_gate)
            if b is not None:
                # last half-chunk split in two so the final dense work does
                # not all land after the last round
                if b == T - 5:
                    p5q.extend(((T - 8) * BL, QN // 2, m) for m in range(MT))
                elif b == T - 1:
                    p5q.extend(((T - 4) * BL, QN // 2, m) for m in range(MT))
                elif b % QS == QS - 1 and b < T - 8:
                    p5q.extend(((b // QS) * QN, QN, m) for m in range(MT))
            for _ in range(1 if r < T else 3):
                if p5q:
                    emit_p5_mm(*p5q.pop(0), after_mm=gate)
            if a is None:
                # no more chain sigmoids to protect: drain evacs immediately
                for unit in p5ev[:]:
                    p5ev.remove(unit)
                    emit_p5_ev(*unit)
        while p5q:
            emit_p5_mm(*p5q.pop(0))
        for unit in p5ev:
            emit_p5_ev(*unit)
    nc.compile()
    return nc


_NC_CACHE = {}
LAST_RESULTS = []  # test harness introspection (exec_time_ns / traces)


def _get_nc():
    if "nc" not in _NC_CACHE:
        _NC_CACHE["nc"] = build_nc()
    return _NC_CACHE["nc"]


def kernel(**inputs):
    x = np.asarray(inputs["x"], np.float32)
    shared = _prep_shared(
        np.asarray(inputs["W0"], np.float32), np.asarray(inputs["U0"], np.float32),
        np.asarray(inputs["b0"], np.float32), np.asarray(inputs["W1"], np.float32),
        np.asarray(inputs["U1"], np.float32), np.asarray(inputs["b1"], np.float32),
        np.asarray(inputs["Wd"], np.float32), np.asarray(inputs["bd"], np.float32))
    in_maps = []
    for c in range(NCORES):
        m = dict(shared)
        m["xT"] = _prep_x(x[c * BL:(c + 1) * BL])
        in_maps.append(m)

    nc = _get_nc()
    res = bass_utils.run_bass_kernel_spmd(nc, in_maps, core_ids=list(range(NCORES)))
    LAST_RESULTS.append(res)

    outs = []
    for c in range(NCORES):
        o = np.asarray(res.results[c]["out"], np.float32)      # (128, MT, NTOK)
        yT = o.transpose(1, 0, 2).reshape(D_OUT_PAD, NTOK)[:D_OUT]
        y = yT.T.reshape(T, BL, N, F_OUT).transpose(1, 2, 0, 3)
        outs.append(y)
    return np.ascontiguousarray(np.concatenate(outs, axis=0), dtype=np.float32)

